# revision 5
# baseline (speedup 1.0000x reference)
"""Trainium2 distributed kernel for nn_ACDE_87531433492502 (moe_routing).

Reference computation (N=65536 pixels, F=224 features, P=6 classes, H=256):
    h      = relu(Y @ W1 + b1); h = relu(h @ W2 + b2); logits = h @ W3 + b3
    cls    = argmax(abundance, -1)
    w      = per-class masked softmax of logits over the pixel axis
    M[p,f] = sum_n w[p,n,f] * Y[n,f]   (0 for empty classes)
    Y_hat  = abundance @ M

Sharding: data-parallel over pixels (8192/core). Each core runs the MLP on its
shard in transposed layout (features on partitions), computes a local
per-feature logit max, AllReduce-max's it (softmax shift only needs a
consistent per-feature offset, not the exact per-class max — the ratio
sy/se is shift-invariant), then forms E = exp(logits - c) and E*Y, xbar-DMA
transposes them to pixel-major, and reduces per-class sums se = Mask^T E and
sy = Mask^T (E*Y) with PE matmuls against the one-hot class mask. One
AllReduce-add of [se;sy] gives the global sums; M = sy/se is formed on every
core (replicated) and the final Y_hat^T = M^T @ A^T GEMM is data-parallel.

Matmuls run in bf16 with f32 PSUM accumulation (validated ~2.8e-3 rel err vs
the f32 reference, tolerance 2e-2).
"""

import sys

for _p in ("/opt/trn_rl_repo", "/root/.axon_site/_ro/trn_rl_repo"):
    if _p not in sys.path:
        sys.path.insert(0, _p)

import numpy as np
import ml_dtypes

N_FULL = 65536
F = 224
P = 6
H = 256
N_CORES = 8
N_LOC = N_FULL // N_CORES

BF16 = ml_dtypes.bfloat16


def build(n_loc=N_LOC, n_cores=N_CORES):
    import concourse.bass as bass
    import concourse.mybir as mybir
    from concourse import bacc
    from concourse import tile
    from contextlib import ExitStack

    dt = mybir.dt
    AF = mybir.ActivationFunctionType
    ALU = mybir.AluOpType

    CH = 512                    # pixels per processing chunk
    nch = n_loc // CH
    ntile = n_loc // 128        # 128-pixel tiles for the class-sum matmuls
    rg = [list(range(n_cores))]

    nc = bacc.Bacc(
        "TRN2",
        target_bir_lowering=False,
        debug=False,
        num_devices=n_cores,
    )

    # ---- external I/O (per-core shards, host-prepped layouts) ----
    yt = nc.dram_tensor("yt", [F, n_loc], dt.bfloat16, kind="ExternalInput")
    at = nc.dram_tensor("at", [P, n_loc], dt.bfloat16, kind="ExternalInput")
    # one-hot mask, pre-tiled on host: [128, ntile*6]; column block t holds
    # the [128, 6] one-hot rows for pixels t*128..t*128+127
    mask = nc.dram_tensor("mask", [128, ntile * P], dt.bfloat16, kind="ExternalInput")
    w1 = nc.dram_tensor("w1", [F, H], dt.bfloat16, kind="ExternalInput")
    w2 = nc.dram_tensor("w2", [H, H], dt.bfloat16, kind="ExternalInput")
    w3 = nc.dram_tensor("w3", [H, F], dt.bfloat16, kind="ExternalInput")
    b1 = nc.dram_tensor("b1", [H, 1], dt.float32, kind="ExternalInput")
    b2 = nc.dram_tensor("b2", [H, 1], dt.float32, kind="ExternalInput")
    b3 = nc.dram_tensor("b3", [F, 1], dt.float32, kind="ExternalInput")
    out = nc.dram_tensor("out", [F, n_loc], dt.float32, kind="ExternalOutput")

    FT = ((128, 0), (96, 128))   # feature-partition tiles of 224
    HT = ((128, 0), (128, 128))  # hidden-partition tiles of 256

    with tile.TileContext(nc) as tc, ExitStack() as ctx:
        consts = ctx.enter_context(tc.tile_pool(name="consts", bufs=1))
        ytp = ctx.enter_context(tc.tile_pool(name="ytp", bufs=1))
        lgp = ctx.enter_context(tc.tile_pool(name="lgp", bufs=1))
        work = ctx.enter_context(tc.tile_pool(name="work", bufs=2))
        rmp = ctx.enter_context(tc.tile_pool(name="rmp", bufs=3))
        pp = ctx.enter_context(tc.tile_pool(name="pp", space="PSUM", bufs=2))
        ppacc = ctx.enter_context(tc.tile_pool(name="ppacc", space="PSUM", bufs=1))
        dram = ctx.enter_context(tc.tile_pool(name="dram", bufs=1, space="DRAM"))

        # ---- collective bounce buffers ----
        mx_in = dram.tile([F, 1], dt.float32, name="mx_in")
        mx_out = dram.tile([F, 1], dt.float32, addr_space="Shared", name="mx_out")
        s_in = dram.tile([2 * P, F], dt.float32, name="s_in")
        s_out = dram.tile([2 * P, F], dt.float32, addr_space="Shared", name="s_out")

        # ---- constants into SBUF ----
        w1_sb = []
        for k, (ks, kofs) in enumerate(FT):
            t = consts.tile([ks, H], dt.bfloat16, name=f"w1_sb{k}")
            nc.sync.dma_start(out=t, in_=w1[kofs:kofs + ks, :])
            w1_sb.append(t)
        w2_sb = []
        for k, (ks, kofs) in enumerate(HT):
            t = consts.tile([ks, H], dt.bfloat16, name=f"w2_sb{k}")
            nc.sync.dma_start(out=t, in_=w2[kofs:kofs + ks, :])
            w2_sb.append(t)
        w3_sb = []
        for k, (ks, kofs) in enumerate(HT):
            t = consts.tile([ks, F], dt.bfloat16, name=f"w3_sb{k}")
            nc.sync.dma_start(out=t, in_=w3[kofs:kofs + ks, :])
            w3_sb.append(t)
        b1_sb = []
        b2_sb = []
        for m, (ms, mofs) in enumerate(HT):
            t1 = consts.tile([ms, 1], dt.float32, name=f"b1_sb{m}")
            nc.sync.dma_start(out=t1, in_=b1[mofs:mofs + ms, :])
            b1_sb.append(t1)
            t2 = consts.tile([ms, 1], dt.float32, name=f"b2_sb{m}")
            nc.sync.dma_start(out=t2, in_=b2[mofs:mofs + ms, :])
            b2_sb.append(t2)
        b3_sb = []
        for m, (ms, mofs) in enumerate(FT):
            t = consts.tile([ms, 1], dt.float32, name=f"b3_sb{m}")
            nc.sync.dma_start(out=t, in_=b3[mofs:mofs + ms, :])
            b3_sb.append(t)
        mask_sb = consts.tile([128, ntile * P], dt.bfloat16, name="mask_sb")
        nc.sync.dma_start(out=mask_sb, in_=mask[:, :])
        at_sb = consts.tile([P, n_loc], dt.bfloat16, name="at_sb")
        nc.sync.dma_start(out=at_sb, in_=at[:, :])

        # local per-feature max partials, one column per chunk
        lmaxp = [
            consts.tile([ms, nch], dt.float32, name=f"lmaxp{m}")
            for m, (ms, _) in enumerate(FT)
        ]

        # persistent per-chunk tiles
        yt_ch = [[None] * nch for _ in range(2)]
        lg_ch = [[None] * nch for _ in range(2)]

        # ================= phase 1: MLP, logits, local max =================
        for c in range(nch):
            sl = slice(c * CH, (c + 1) * CH)
            for m, (ms, mofs) in enumerate(FT):
                t = ytp.tile([ms, CH], dt.bfloat16, name=f"yt{m}_{c}", tag=f"yt{m}_{c}")
                nc.sync.dma_start(out=t, in_=yt[mofs:mofs + ms, sl])
                yt_ch[m][c] = t

            h1 = []
            for m, (ms, mofs) in enumerate(HT):
                ps = pp.tile([128, CH], dt.float32, name=f"h1ps_{m}_{c}", tag="h1ps")
                nc.tensor.matmul(ps, w1_sb[0][:, mofs:mofs + ms], yt_ch[0][c],
                                 start=True, stop=False)
                nc.tensor.matmul(ps, w1_sb[1][:, mofs:mofs + ms], yt_ch[1][c],
                                 start=False, stop=True)
                ht = work.tile([128, CH], dt.bfloat16, name=f"h1_{m}_{c}", tag=f"h1_{m}")
                nc.scalar.activation(ht, ps, AF.Relu, bias=b1_sb[m], scale=1.0)
                h1.append(ht)

            h2 = []
            for m, (ms, mofs) in enumerate(HT):
                ps = pp.tile([128, CH], dt.float32, name=f"h2ps_{m}_{c}", tag="h2ps")
                nc.tensor.matmul(ps, w2_sb[0][:, mofs:mofs + ms], h1[0],
                                 start=True, stop=False)
                nc.tensor.matmul(ps, w2_sb[1][:, mofs:mofs + ms], h1[1],
                                 start=False, stop=True)
                ht = work.tile([128, CH], dt.bfloat16, name=f"h2_{m}_{c}", tag=f"h2_{m}")
                nc.scalar.activation(ht, ps, AF.Relu, bias=b2_sb[m], scale=1.0)
                h2.append(ht)

            for m, (ms, mofs) in enumerate(FT):
                ps = pp.tile([128, CH], dt.float32, name=f"lgps_{m}_{c}", tag="lgps")
                nc.tensor.matmul(ps[0:ms], w3_sb[0][:, mofs:mofs + ms], h2[0],
                                 start=True, stop=False)
                nc.tensor.matmul(ps[0:ms], w3_sb[1][:, mofs:mofs + ms], h2[1],
                                 start=False, stop=True)
                lg = lgp.tile([ms, CH], dt.float32, name=f"lg{m}_{c}", tag=f"lg{m}_{c}")
                # logits = psum + b3 (f32, kept in SBUF for the exp after AR1)
                nc.vector.tensor_scalar(lg, ps[0:ms], b3_sb[m], None, op0=ALU.add)
                lg_ch[m][c] = lg
                # local per-feature max partial (free-axis reduce -> DVE)
                nc.vector.tensor_reduce(
                    out=lmaxp[m][:, c:c + 1], in_=lg, op=ALU.max,
                    axis=mybir.AxisListType.X,
                )

        # ================= AllReduce #1: per-feature max =================
        lmax = []
        for m, (ms, mofs) in enumerate(FT):
            t = consts.tile([ms, 1], dt.float32, name=f"lmax{m}")
            nc.vector.tensor_reduce(out=t, in_=lmaxp[m], op=ALU.max,
                                    axis=mybir.AxisListType.X)
            nc.sync.dma_start(out=mx_in[mofs:mofs + ms, :], in_=t)
            lmax.append(t)

        nc.gpsimd.collective_compute(
            "AllReduce", ALU.max, replica_groups=rg,
            ins=[mx_in.opt()], outs=[mx_out.opt()],
        )

        ngmax = []
        for m, (ms, mofs) in enumerate(FT):
            g = consts.tile([ms, 1], dt.float32, name=f"gmax{m}")
            nc.sync.dma_start(out=g, in_=mx_out[mofs:mofs + ms, :])
            ng = consts.tile([ms, 1], dt.float32, name=f"ngmax{m}")
            nc.vector.tensor_scalar(ng, g, -1.0, None, op0=ALU.mult)
            ngmax.append(ng)

        # ============ phase 2: E, E*Y, transpose, class sums ============
        se_ps = ppacc.tile([P, F], dt.float32, name="se_ps", tag="se_ps")
        sy_ps = ppacc.tile([P, F], dt.float32, name="sy_ps", tag="sy_ps")

        for c in range(nch):
            et = []
            eyt = []
            for m, (ms, mofs) in enumerate(FT):
                e = work.tile([ms, CH], dt.bfloat16, name=f"et{m}_{c}", tag=f"et{m}")
                nc.scalar.activation(e, lg_ch[m][c], AF.Exp, bias=ngmax[m], scale=1.0)
                et.append(e)
                ey = work.tile([ms, CH], dt.bfloat16, name=f"eyt{m}_{c}", tag=f"eyt{m}")
                nc.vector.tensor_tensor(out=ey, in0=e, in1=yt_ch[m][c], op=ALU.mult)
                eyt.append(ey)

            for i in range(CH // 128):
                t_glob = c * (CH // 128) + i
                erm = rmp.tile([128, F], dt.bfloat16, name=f"erm_{t_glob}", tag="erm")
                eyrm = rmp.tile([128, F], dt.bfloat16, name=f"eyrm_{t_glob}", tag="eyrm")
                for m, (ms, mofs) in enumerate(FT):
                    nc.sync.dma_start(
                        out=erm[:, mofs:mofs + ms],
                        in_=et[m][:, i * 128:(i + 1) * 128],
                        transpose=True,
                    )
                    nc.sync.dma_start(
                        out=eyrm[:, mofs:mofs + ms],
                        in_=eyt[m][:, i * 128:(i + 1) * 128],
                        transpose=True,
                    )
                msl = mask_sb[:, t_glob * P:(t_glob + 1) * P]
                first = t_glob == 0
                last = t_glob == ntile - 1
                nc.tensor.matmul(se_ps, msl, erm, start=first, stop=last,
                                 skip_group_check=True)
                nc.tensor.matmul(sy_ps, msl, eyrm, start=first, stop=last,
                                 skip_group_check=True)

        # ================= AllReduce #2: se / sy sums =================
        se_sb = consts.tile([P, F], dt.float32, name="se_sb")
        sy_sb = consts.tile([P, F], dt.float32, name="sy_sb")
        nc.vector.tensor_copy(out=se_sb, in_=se_ps)
        nc.vector.tensor_copy(out=sy_sb, in_=sy_ps)
        nc.sync.dma_start(out=s_in[0:P, :], in_=se_sb)
        nc.sync.dma_start(out=s_in[P:2 * P, :], in_=sy_sb)

        nc.gpsimd.collective_compute(
            "AllReduce", ALU.add, replica_groups=rg,
            ins=[s_in.opt()], outs=[s_out.opt()],
        )

        seg = consts.tile([P, F], dt.float32, name="seg")
        syg = consts.tile([P, F], dt.float32, name="syg")
        nc.sync.dma_start(out=seg, in_=s_out[0:P, :])
        nc.sync.dma_start(out=syg, in_=s_out[P:2 * P, :])
        # M = syg / max(seg, tiny); empty classes give syg==0 -> M==0 exactly
        rec = consts.tile([P, F], dt.float32, name="rec")
        nc.vector.tensor_scalar(rec, seg, 1e-30, None, op0=ALU.max)
        nc.vector.reciprocal(out=rec, in_=rec)
        m_bf = consts.tile([P, F], dt.bfloat16, name="m_bf")
        nc.vector.tensor_tensor(out=m_bf, in0=syg, in1=rec, op=ALU.mult)

        # ============ phase 3: Y_hat^T = M^T @ A^T (data-parallel) ============
        for c in range(nch):
            sl = slice(c * CH, (c + 1) * CH)
            for m, (ms, mofs) in enumerate(FT):
                ps = pp.tile([128, CH], dt.float32, name=f"ops_{m}_{c}", tag="h1ps")
                nc.tensor.matmul(ps[0:ms], m_bf[:, mofs:mofs + ms], at_sb[:, sl],
                                 start=True, stop=True)
                osb = work.tile([ms, CH], dt.float32, name=f"osb_{m}_{c}", tag=f"osb{m}")
                if m == 0:
                    nc.scalar.copy(out=osb, in_=ps[0:ms])
                else:
                    nc.vector.tensor_copy(out=osb, in_=ps[0:ms])
                nc.sync.dma_start(out=out[mofs:mofs + ms, sl], in_=osb)

    nc.compile()
    return nc


_CACHE = {}


def _get_nc(n_loc):
    if n_loc not in _CACHE:
        _CACHE[n_loc] = build(n_loc)
    return _CACHE[n_loc]


def make_in_maps(inputs, n_loc=N_LOC, n_cores=N_CORES):
    """Shard + lay out the full inputs for the SPMD kernel."""
    A = np.asarray(inputs["abundance_matrix"], dtype=np.float32)
    Y = np.asarray(inputs["Y"], dtype=np.float32)
    W1 = np.asarray(inputs["W1"], dtype=np.float32).astype(BF16)
    W2 = np.asarray(inputs["W2"], dtype=np.float32).astype(BF16)
    W3 = np.asarray(inputs["W3"], dtype=np.float32).astype(BF16)
    b1 = np.asarray(inputs["b1"], dtype=np.float32).reshape(-1, 1)
    b2 = np.asarray(inputs["b2"], dtype=np.float32).reshape(-1, 1)
    b3 = np.asarray(inputs["b3"], dtype=np.float32).reshape(-1, 1)

    cls = np.argmax(A, axis=1)
    onehot = (cls[:, None] == np.arange(P)[None, :]).astype(BF16)

    ntile = n_loc // 128
    in_maps = []
    for i in range(n_cores):
        sl = slice(i * n_loc, (i + 1) * n_loc)
        yt = np.ascontiguousarray(Y[sl].T).astype(BF16)
        at = np.ascontiguousarray(A[sl].T).astype(BF16)
        mask_t = np.ascontiguousarray(
            onehot[sl].reshape(ntile, 128, P).transpose(1, 0, 2).reshape(128, ntile * P)
        )
        in_maps.append({
            "yt": yt, "at": at, "mask": mask_t,
            "w1": W1, "w2": W2, "w3": W3,
            "b1": b1, "b2": b2, "b3": b3,
        })
    return in_maps


def kernel(**inputs):
    from concourse.bass_utils import run_bass_kernel_spmd

    n_loc = N_LOC
    n_cores = N_CORES
    nc = _get_nc(n_loc)
    in_maps = make_in_maps(inputs, n_loc, n_cores)
    res = run_bass_kernel_spmd(nc, in_maps, core_ids=list(range(n_cores)))
    out = np.empty((N_FULL, F), dtype=np.float32)
    for i in range(n_cores):
        out[i * n_loc:(i + 1) * n_loc] = res.results[i]["out"].T
    return out


if __name__ == "__main__":
    import importlib.util

    spec = importlib.util.spec_from_file_location("reference", "/root/problem/reference.py")
    ref = importlib.util.module_from_spec(spec)
    spec.loader.exec_module(ref)
    inputs = {k: np.asarray(v) for k, v in ref.setup_inputs().items()}
    got = kernel(**inputs)
    exp = np.asarray(ref.reference(**inputs))
    rel = np.linalg.norm(got - exp) / np.linalg.norm(exp)
    print("Relative error:", rel)


# revision 14
# speedup vs baseline: 2.5779x; 2.5779x over previous
"""Trainium2 distributed kernel for nn_ACDE_87531433492502 (moe_routing).

Reference (N=65536 pixels, F=224 features, P=6 classes, H=256):
    h      = relu(Y @ W1 + b1); h = relu(h @ W2 + b2); logits = h @ W3 + b3
    cls    = argmax(abundance, -1)
    w      = per-class masked softmax of logits over the pixel axis
    M[p,f] = sum_n w[p,n,f] * Y[n,f]   (0 for empty classes)
    Y_hat  = abundance @ M

Distribution strategy (8 cores, data-parallel over pixels):
  * Pixels are sorted by class on the host and dealt round-robin so every
    core holds the same per-class segment structure [L_0..L_5] (pad slots get
    Y=0 / A=0). The per-class masked softmax sums then become contiguous
    free-axis segment reductions in the transposed (feature-major) layout —
    no on-device transposes or mask matmuls are needed.
  * MLP runs in transposed layout (features on partitions, pixels on the free
    axis) in bf16 with f32 PSUM accumulation.
  * Softmax shift: a per-feature LOCAL max is used for exp (safe: values <=1)
    so the max-AllReduce overlaps phase-2 compute; the local sums are then
    rescaled by exp(localmax - globalmax) before the sum-AllReduce. The
    softmax ratio sy/se is shift-invariant, so any consistent shift is exact.
    (b3 shifts logits per-feature uniformly over pixels, so it cancels in the
    softmax and is not applied on device.)
  * Pad slots flow through the MLP (Y=0 => they add k_p * exp(lpad - c) to
    se and exactly 0 to sy); their se contribution is subtracted exactly
    using an actually-computed pad column of E and the host-known pad counts.
  * One AllReduce-max ([224] f32) + one AllReduce-add ([224,12] f32); the
    replicated M = sy/se feeds the data-parallel Y_hat^T = M^T A^T GEMM.
"""

import sys
from math import ceil

for _p in ("/opt/trn_rl_repo", "/root/.axon_site/_ro/trn_rl_repo"):
    if _p not in sys.path:
        sys.path.insert(0, _p)

import numpy as np
import ml_dtypes

N_FULL = 65536
F = 224
P = 6
H = 256
N_CORES = 8
CH = 512  # pixels per MLP chunk
GRP = 4   # chunks per weight-stationary group

BF16 = ml_dtypes.bfloat16


USE_ACT_ACCUM = True   # se via exp's accum_out (verified on HW)
USE_TTR = False        # tensor_tensor_reduce crashes the device — keep off
EPAD_F32 = True        # copy pad column to f32 before scalar_tensor_tensor


def build(n_loc, segs, n_cores=N_CORES):
    """Build the SPMD graph. n_loc = padded pixels/core, segs = per-class
    segment lengths (identical on every core)."""
    import concourse.bass as bass  # noqa: F401
    import concourse.mybir as mybir
    from concourse import bacc
    from concourse import tile
    from contextlib import ExitStack

    dt = mybir.dt
    AF = mybir.ActivationFunctionType
    ALU = mybir.AluOpType

    nch = n_loc // CH
    offs = [0]
    for L in segs:
        offs.append(offs[-1] + L)
    assert offs[-1] < n_loc  # at least one guaranteed tail pad column
    rg = [list(range(n_cores))]

    nc = bacc.Bacc(
        "TRN2", target_bir_lowering=False, debug=False, num_devices=n_cores
    )

    # ---- external I/O ----
    yt = nc.dram_tensor("yt", [F, n_loc], dt.bfloat16, kind="ExternalInput")
    at = nc.dram_tensor("at", [P, n_loc], dt.bfloat16, kind="ExternalInput")
    w1 = nc.dram_tensor("w1", [F, H], dt.bfloat16, kind="ExternalInput")
    w2 = nc.dram_tensor("w2", [H, H], dt.bfloat16, kind="ExternalInput")
    w3 = nc.dram_tensor("w3", [H, F], dt.bfloat16, kind="ExternalInput")
    b1 = nc.dram_tensor("b1", [H, 1], dt.float32, kind="ExternalInput")
    b2 = nc.dram_tensor("b2", [H, 1], dt.float32, kind="ExternalInput")
    kneg = nc.dram_tensor("kneg", [128, P], dt.float32, kind="ExternalInput")
    ident = nc.dram_tensor("ident", [128, 128], dt.bfloat16, kind="ExternalInput")
    out = nc.dram_tensor("out", [F, n_loc], dt.float32, kind="ExternalOutput")

    FT = ((128, 0), (96, 128))   # feature-partition tiles of 224
    HT = ((128, 0), (128, 128))  # hidden-partition tiles of 256

    with tile.TileContext(nc) as tc, ExitStack() as ctx:
        consts = ctx.enter_context(tc.tile_pool(name="consts", bufs=1))
        big = ctx.enter_context(tc.tile_pool(name="big", bufs=1))
        work = ctx.enter_context(tc.tile_pool(name="work", bufs=4))
        prodp = ctx.enter_context(tc.tile_pool(name="prodp", bufs=2))
        pp = ctx.enter_context(tc.tile_pool(name="pp", space="PSUM", bufs=8))
        dram = ctx.enter_context(tc.tile_pool(name="dram", bufs=1, space="DRAM"))

        # ---- collective bounce buffers ----
        mx_in = dram.tile([F, 1], dt.float32, name="mx_in")
        mx_out = dram.tile([F, 1], dt.float32, addr_space="Shared", name="mx_out")
        s_in = dram.tile([F, 2 * P], dt.float32, name="s_in")
        s_out = dram.tile([F, 2 * P], dt.float32, addr_space="Shared", name="s_out")

        # ---- constants ----
        def load_w(name, w_dram, tiles):
            sb = []
            for k, (ks, kofs) in enumerate(tiles):
                t = consts.tile([ks, w_dram.shape[1]], dt.bfloat16, name=f"{name}{k}")
                nc.sync.dma_start(out=t, in_=w_dram[kofs:kofs + ks, :])
                sb.append(t)
            return sb

        w1_sb = load_w("w1sb", w1, FT)
        w2_sb = load_w("w2sb", w2, HT)
        w3_sb = load_w("w3sb", w3, HT)
        b1_sb = []
        b2_sb = []
        for m, (ms, mofs) in enumerate(HT):
            t1 = consts.tile([ms, 1], dt.float32, name=f"b1sb{m}")
            nc.sync.dma_start(out=t1, in_=b1[mofs:mofs + ms, :])
            b1_sb.append(t1)
            t2 = consts.tile([ms, 1], dt.float32, name=f"b2sb{m}")
            nc.sync.dma_start(out=t2, in_=b2[mofs:mofs + ms, :])
            b2_sb.append(t2)
        kneg_sb = consts.tile([128, P], dt.float32, name="kneg_sb")
        nc.sync.dma_start(out=kneg_sb, in_=kneg[:, :])
        ident_sb = consts.tile([128, 128], dt.bfloat16, name="ident_sb")
        nc.sync.dma_start(out=ident_sb, in_=ident[:, :])
        at_sb = consts.tile([P, n_loc], dt.bfloat16, name="at_sb")
        nc.sync.dma_start(out=at_sb, in_=at[:, :])

        # ---- big persistent tensors (slice-written) ----
        yt_sb = [big.tile([ms, n_loc], dt.bfloat16, name=f"ytsb{m}")
                 for m, (ms, _) in enumerate(FT)]
        lg_sb = [big.tile([ms, n_loc], dt.float32, name=f"lgsb{m}")
                 for m, (ms, _) in enumerate(FT)]
        et_sb = [big.tile([ms, n_loc], dt.bfloat16, name=f"etsb{m}")
                 for m, (ms, _) in enumerate(FT)]

        for c in range(nch):
            sl = slice(c * CH, (c + 1) * CH)
            for m, (ms, mofs) in enumerate(FT):
                nc.sync.dma_start(out=yt_sb[m][:, sl], in_=yt[mofs:mofs + ms, sl])

        # ================= phase 1: MLP into lg (weight-stationary groups) ====
        for g0 in range(0, nch, GRP):
            cs = range(g0, min(g0 + GRP, nch))
            sls = [slice(c * CH, (c + 1) * CH) for c in cs]

            # L1: h1 = relu(W1^T @ Y + b1)
            h1 = {}
            ps1 = {}
            for m, (ms, mofs) in enumerate(HT):
                for ci, sl in zip(cs, sls):
                    ps1[m, ci] = pp.tile([128, CH], dt.float32,
                                         name=f"ps1_{m}_{ci}", tag="ps")
                for k in range(2):
                    lhs = w1_sb[k][:, mofs:mofs + ms]
                    for ci, sl in zip(cs, sls):
                        nc.tensor.matmul(ps1[m, ci], lhs, yt_sb[k][:, sl],
                                         start=(k == 0), stop=(k == 1))
                for ci, sl in zip(cs, sls):
                    ht = work.tile([128, CH], dt.bfloat16,
                                   name=f"h1_{m}_{ci}", tag=f"h1_{m}")
                    nc.scalar.activation(ht, ps1[m, ci], AF.Relu,
                                         bias=b1_sb[m], scale=1.0)
                    h1[m, ci] = ht

            # L2: h2 = relu(W2^T @ h1 + b2)   (evac on DVE to balance engines)
            h2 = {}
            ps2 = {}
            for m, (ms, mofs) in enumerate(HT):
                for ci in cs:
                    ps2[m, ci] = pp.tile([128, CH], dt.float32,
                                         name=f"ps2_{m}_{ci}", tag="ps")
                for k in range(2):
                    lhs = w2_sb[k][:, mofs:mofs + ms]
                    for ci in cs:
                        nc.tensor.matmul(ps2[m, ci], lhs, h1[k, ci],
                                         start=(k == 0), stop=(k == 1))
                for ci in cs:
                    ht = work.tile([128, CH], dt.bfloat16,
                                   name=f"h2_{m}_{ci}", tag=f"h2_{m}")
                    nc.vector.tensor_scalar(ht, ps2[m, ci], b2_sb[m], 0.0,
                                            op0=ALU.add, op1=ALU.max)
                    h2[m, ci] = ht

            # L3: logits = W3^T @ h2  (b3 cancels in the softmax; ACT evac)
            for m, (ms, mofs) in enumerate(FT):
                ps3 = {}
                for ci in cs:
                    ps3[ci] = pp.tile([128, CH], dt.float32,
                                      name=f"ps3_{m}_{ci}", tag="ps")
                for k in range(2):
                    lhs = w3_sb[k][:, mofs:mofs + ms]
                    for ci in cs:
                        nc.tensor.matmul(ps3[ci][0:ms], lhs, h2[k, ci],
                                         start=(k == 0), stop=(k == 1))
                for ci in cs:
                    sl = slice(ci * CH, (ci + 1) * CH)
                    nc.scalar.copy(out=lg_sb[m][:, sl], in_=ps3[ci][0:ms])

        # ======== local per-feature max; AllReduce-max runs overlapped ========
        lmax = []
        nlmax = []
        for m, (ms, mofs) in enumerate(FT):
            t = consts.tile([ms, 1], dt.float32, name=f"lmax{m}")
            nc.vector.tensor_reduce(out=t, in_=lg_sb[m], op=ALU.max,
                                    axis=mybir.AxisListType.X)
            lmax.append(t)
            ng = consts.tile([ms, 1], dt.float32, name=f"nlmax{m}")
            nc.vector.tensor_scalar(ng, t, -1.0, None, op0=ALU.mult)
            nlmax.append(ng)
            nc.gpsimd.dma_start(out=mx_in[mofs:mofs + ms, :], in_=t)

        nc.gpsimd.collective_compute(
            "AllReduce", ALU.max, replica_groups=rg,
            ins=[mx_in.opt()], outs=[mx_out.opt()],
        )

        # ============ phase 2: E = exp(lg - lmax), segment sums ============
        se_t = [consts.tile([ms, P], dt.float32, name=f"se_t{m}")
                for m, (ms, _) in enumerate(FT)]
        sy_t = [consts.tile([ms, P], dt.float32, name=f"sy_t{m}")
                for m, (ms, _) in enumerate(FT)]
        for m in range(2):
            nc.vector.memset(se_t[m], 0.0)
            nc.vector.memset(sy_t[m], 0.0)

        for m, (ms, mofs) in enumerate(FT):
            for p in range(P):
                if segs[p] == 0:
                    continue
                seg = slice(offs[p], offs[p + 1])
                if USE_ACT_ACCUM:
                    nc.scalar.activation(
                        out=et_sb[m][:, seg], in_=lg_sb[m][:, seg], func=AF.Exp,
                        bias=nlmax[m], scale=1.0,
                        accum_out=se_t[m][:, p:p + 1],
                    )
                else:
                    nc.scalar.activation(
                        out=et_sb[m][:, seg], in_=lg_sb[m][:, seg], func=AF.Exp,
                        bias=nlmax[m], scale=1.0,
                    )
                    nc.vector.tensor_reduce(
                        out=se_t[m][:, p:p + 1], in_=et_sb[m][:, seg],
                        op=ALU.add, axis=mybir.AxisListType.X,
                    )
                prod = prodp.tile([ms, segs[p]], dt.bfloat16,
                                  name=f"prod{m}_{p}", tag=f"prod{m}")
                if USE_TTR:
                    nc.vector.tensor_tensor_reduce(
                        out=prod, in0=et_sb[m][:, seg], in1=yt_sb[m][:, seg],
                        scale=1.0, scalar=0.0, op0=ALU.mult, op1=ALU.add,
                        accum_out=sy_t[m][:, p:p + 1],
                    )
                else:
                    nc.vector.tensor_tensor(out=prod, in0=et_sb[m][:, seg],
                                            in1=yt_sb[m][:, seg], op=ALU.mult)
                    nc.vector.tensor_reduce(
                        out=sy_t[m][:, p:p + 1], in_=prod,
                        op=ALU.add, axis=mybir.AxisListType.X,
                    )
            # guaranteed pad column right after the last segment
            nc.scalar.activation(
                out=et_sb[m][:, offs[-1]:offs[-1] + 1],
                in_=lg_sb[m][:, offs[-1]:offs[-1] + 1],
                func=AF.Exp, bias=nlmax[m], scale=1.0,
            )

        # pad correction: se -= k_p * E_pad  (exact, E_pad actually computed)
        se_c = [consts.tile([ms, P], dt.float32, name=f"se_c{m}")
                for m, (ms, _) in enumerate(FT)]
        for m, (ms, mofs) in enumerate(FT):
            epad = et_sb[m][:, offs[-1]:offs[-1] + 1]
            if EPAD_F32:
                ep32 = consts.tile([ms, 1], dt.float32, name=f"ep32_{m}")
                nc.vector.tensor_copy(out=ep32, in_=epad)
                epad = ep32
            nc.vector.scalar_tensor_tensor(
                out=se_c[m], in0=kneg_sb[0:ms, :],
                scalar=epad, in1=se_t[m],
                op0=ALU.mult, op1=ALU.add,
            )

        # global max arrives; rescale local sums by exp(lmax - gmax)
        se_s = [consts.tile([ms, P], dt.float32, name=f"se_s{m}")
                for m, (ms, _) in enumerate(FT)]
        sy_s = [consts.tile([ms, P], dt.float32, name=f"sy_s{m}")
                for m, (ms, _) in enumerate(FT)]
        for m, (ms, mofs) in enumerate(FT):
            g = consts.tile([ms, 1], dt.float32, name=f"gmax{m}")
            nc.gpsimd.dma_start(out=g, in_=mx_out[mofs:mofs + ms, :])
            r = consts.tile([ms, 1], dt.float32, name=f"rfac{m}")
            nc.scalar.activation(out=r, in_=g, func=AF.Exp,
                                 bias=lmax[m], scale=-1.0)
            nc.vector.tensor_scalar(se_s[m], se_c[m], r, None, op0=ALU.mult)
            nc.vector.tensor_scalar(sy_s[m], sy_t[m], r, None, op0=ALU.mult)
            nc.sync.dma_start(out=s_in[mofs:mofs + ms, 0:P], in_=se_s[m])
            nc.sync.dma_start(out=s_in[mofs:mofs + ms, P:2 * P], in_=sy_s[m])

        # ================= AllReduce #2: global se / sy =================
        nc.gpsimd.collective_compute(
            "AllReduce", ALU.add, replica_groups=rg,
            ins=[s_in.opt()], outs=[s_out.opt()],
        )

        # M^T = sy/se in feature-major layout, then PE-transpose to [P, F]
        m_ps = pp.tile([P, F], dt.bfloat16, name="m_ps", tag="ps")
        for m, (ms, mofs) in enumerate(FT):
            srt = consts.tile([ms, 2 * P], dt.float32, name=f"srt{m}")
            nc.sync.dma_start(out=srt, in_=s_out[mofs:mofs + ms, :])
            rec = consts.tile([ms, P], dt.float32, name=f"rec{m}")
            nc.vector.tensor_scalar(rec, srt[:, 0:P], 1e-30, None, op0=ALU.max)
            nc.vector.reciprocal(out=rec, in_=rec)
            mt = consts.tile([ms, P], dt.bfloat16, name=f"mt{m}")
            nc.vector.tensor_tensor(out=mt, in0=srt[:, P:2 * P], in1=rec,
                                    op=ALU.mult)
            nc.tensor.transpose(m_ps[:, mofs:mofs + ms], mt, ident_sb[0:ms, 0:ms])
        m_sb = consts.tile([P, F], dt.bfloat16, name="m_sb")
        nc.vector.tensor_copy(out=m_sb, in_=m_ps)

        # ============ phase 3: Y_hat^T = M^T @ A^T (data-parallel) ============
        for c in range(nch):
            sl = slice(c * CH, (c + 1) * CH)
            for m, (ms, mofs) in enumerate(FT):
                ps = pp.tile([128, CH], dt.float32, name=f"ops_{m}_{c}", tag="ps")
                nc.tensor.matmul(ps[0:ms], m_sb[:, mofs:mofs + ms], at_sb[:, sl],
                                 start=True, stop=True)
                osb = work.tile([ms, CH], dt.float32, name=f"osb_{m}_{c}",
                                tag=f"osb{m}")
                if m == 0:
                    nc.scalar.copy(out=osb, in_=ps[0:ms])
                else:
                    nc.vector.tensor_copy(out=osb, in_=ps[0:ms])
                nc.sync.dma_start(out=out[mofs:mofs + ms, sl], in_=osb)

    nc.compile()
    return nc


_CACHE = {}


def _get_nc(n_loc, segs):
    key = (n_loc, tuple(segs))
    if key not in _CACHE:
        _CACHE[key] = build(n_loc, tuple(segs))
    return _CACHE[key]


def prepare(inputs, n_cores=N_CORES):
    """Class-sort + balance-deal pixels; build per-core inputs and metadata."""
    A = np.asarray(inputs["abundance_matrix"], dtype=np.float32)
    Y = np.asarray(inputs["Y"], dtype=np.float32)
    n_full = A.shape[0]

    cls = np.argmax(A, axis=1)
    idx_p = [np.flatnonzero(cls == p) for p in range(P)]
    segs = [len(ix[0::n_cores]) for ix in idx_p]  # ceil(count/n_cores)
    n_loc = CH * ceil((sum(segs) + 1) / CH)       # >=1 guaranteed tail pad
    offs = np.concatenate(([0], np.cumsum(segs)))

    src = np.full((n_cores, n_loc), -1, dtype=np.int64)
    kcore = np.zeros((n_cores, P), np.float32)
    for p in range(P):
        for i in range(n_cores):
            mine = idx_p[p][i::n_cores]
            src[i, offs[p]:offs[p] + len(mine)] = mine
            kcore[i, p] = segs[p] - len(mine)

    W1 = np.asarray(inputs["W1"], dtype=np.float32).astype(BF16)
    W2 = np.asarray(inputs["W2"], dtype=np.float32).astype(BF16)
    W3 = np.asarray(inputs["W3"], dtype=np.float32).astype(BF16)
    b1 = np.asarray(inputs["b1"], dtype=np.float32).reshape(-1, 1)
    b2 = np.asarray(inputs["b2"], dtype=np.float32).reshape(-1, 1)
    ident = np.eye(128, dtype=BF16)

    in_maps = []
    for i in range(n_cores):
        sel = src[i]
        valid = sel >= 0
        Yl = np.zeros((n_loc, Y.shape[1]), np.float32)
        Yl[valid] = Y[sel[valid]]
        Al = np.zeros((n_loc, P), np.float32)
        Al[valid] = A[sel[valid]]
        in_maps.append({
            "yt": np.ascontiguousarray(Yl.T).astype(BF16),
            "at": np.ascontiguousarray(Al.T).astype(BF16),
            "w1": W1, "w2": W2, "w3": W3, "b1": b1, "b2": b2,
            "kneg": np.tile(-kcore[i], (128, 1)).astype(np.float32),
            "ident": ident,
        })
    return in_maps, src, segs, n_loc, n_full


def kernel(**inputs):
    from concourse.bass_utils import run_bass_kernel_spmd

    in_maps, src, segs, n_loc, n_full = prepare(inputs)
    nc = _get_nc(n_loc, segs)
    res = run_bass_kernel_spmd(nc, in_maps, core_ids=list(range(N_CORES)))
    out = np.empty((n_full, F), dtype=np.float32)
    for i in range(N_CORES):
        sel = src[i]
        valid = sel >= 0
        out[sel[valid]] = res.results[i]["out"].T[valid]
    return out


if __name__ == "__main__":
    import importlib.util

    spec = importlib.util.spec_from_file_location("reference", "/root/problem/reference.py")
    ref = importlib.util.module_from_spec(spec)
    spec.loader.exec_module(ref)
    inputs = {k: np.asarray(v) for k, v in ref.setup_inputs().items()}
    got = kernel(**inputs)
    exp = np.asarray(ref.reference(**inputs))
    rel = np.linalg.norm(got - exp) / np.linalg.norm(exp)
    print("Relative error:", rel)
